# revision 27
# baseline (speedup 1.0000x reference)
"""Trainium2 Bass kernel for a gaussian-moment energy model (GNN message passing).

Strategy (8 NeuronCores, SPMD single program):
  - Host: partition atoms into 8 contiguous shards of 12500 (padded to 12544
    = 98 blocks of 128). Edges owned by the receiver atom i. Sort edges by i,
    group into 32-atom windows, pad each window's edge list to a multiple of
    128 (KT tiles of 128 edges per window, KT = global max). Pre-gather
    R[i]/R[j] per edge slot (the "halo gather" of the sharding hint) and the
    per-edge one-hot window-assignment matrix T (exact 0/1 in bf16).
  - Device per core, single fused pipeline over superchunks of 4 atom-blocks
    (512 atoms): edge pipeline (dr, r, unit dir, 16 gaussian basis fns via one
    broadcast-subtract + Square + Exp, b*d features) -> one-hot scatter
    matmuls into PSUM (per-atom M0/M1 moments) -> j-outer symmetric
    gaussian-moment contraction in bf16 (2x DVE mode) -> PE transpose ->
    3-layer silu MLP (layer-1 bias folded in as a constant-1 feature) ->
    per-atom scale/shift -> scalar sum. Features stay in SBUF end to end.
  - Host: sum the 8 per-core partial energies.
"""

import math
import numpy as np
import ml_dtypes

# ---------------------------------------------------------------- constants
N_ATOMS = 100_000
N_EDGES = 1_600_000
N_RADIAL = 16
HIDDEN = 512
EPS = 1e-8

N_CORES = 8
APC = 12_500          # real atoms per core
# Atoms are bin-packed into windows by degree so every 32-atom window fits in
# KT_PACK*128 edge slots; the extra atom padding buys the packing slack.
APC_PAD = 13_056      # 102 blocks of 128
WIN = 32              # atoms per scatter window (psum col-group)
NWIN = APC_PAD // WIN  # 408
NBLK = APC_PAD // 128  # 102
KT_PACK = 4           # tiles per window when packing succeeds
NPAIR = 144           # 9 j x 16 k1 symmetric-pair slots (j-major)
NFEAT = 16 + NPAIR + 1  # 161: M0, G1, constant-1 (carries the layer-1 bias)
SC_BLK = 8            # max atom blocks per superchunk
SC_BLOCKS = [8] * 12 + [6]
assert sum(SC_BLOCKS) == NBLK
EMAT_P = len(SC_BLOCKS)  # rows in the readout matrix (one per superchunk)


def _preprocess(R, Z, idx, centers, width, W1, b1, W2, b2, W3, b3, scale, shift,
                box=None, offsets=None):
    """Host-side graph partitioning / sharding. Index manipulation + gathers
    only (plus compile-time constant folding of the MLP weights)."""
    R = np.asarray(R, np.float32)
    Z = np.asarray(Z)
    idx_i = np.asarray(idx[0]).astype(np.int64)
    idx_j = np.asarray(idx[1]).astype(np.int64)
    centers = np.asarray(centers, np.float32)
    width = float(np.asarray(width))

    order = np.argsort(idx_i, kind="stable")
    si = idx_i[order]
    sj = idx_j[order]

    core = si // APC                      # owner core per edge
    la = si - core * APC                  # local atom id [0, APC)

    # --- degree-balanced bin packing of atoms into windows (per core) ---
    # Worst-fit decreasing on edge count, respecting <=WIN atoms and
    # <=KT_PACK*128 edges per window. On success every window needs only
    # KT_PACK tiles; on failure fall back to more tiles per window.
    import heapq
    cap = KT_PACK * 128
    # position of each local atom: window id and column within window
    awin = np.zeros((N_CORES, APC), np.int32)
    acol = np.zeros((N_CORES, APC), np.int32)
    kt = KT_PACK
    for c in range(N_CORES):
        deg = np.bincount(la[core == c], minlength=APC)
        order_a = np.argsort(-deg, kind="stable")
        heap = [(0, 0, w) for w in range(NWIN)]   # (edges_used, atoms_used, w)
        ok = True
        spill = []
        for a in order_a:
            d = int(deg[a])
            e_used, n_used, wdx = heap[0]
            if e_used + d > cap or n_used >= WIN:
                # try a few more candidates before declaring failure
                tmp = []
                placed = False
                for _ in range(min(64, len(heap))):
                    e2, n2, w2 = heapq.heappop(heap)
                    if e2 + d <= cap and n2 < WIN:
                        awin[c, a] = w2
                        acol[c, a] = n2
                        heapq.heappush(heap, (e2 + d, n2 + 1, w2))
                        placed = True
                        break
                    tmp.append((e2, n2, w2))
                for t in tmp:
                    heapq.heappush(heap, t)
                if not placed:
                    ok = False
                    spill.append(a)
                continue
            heapq.heappop(heap)
            awin[c, a] = wdx
            acol[c, a] = n_used
            heapq.heappush(heap, (e_used + d, n_used + 1, wdx))
        if not ok:
            # extremely unlikely; place spill atoms wherever atoms fit and
            # grow kt to cover the worst window
            loads = np.zeros(NWIN, np.int64)
            nat = np.zeros(NWIN, np.int64)
            for a in range(APC):
                if a in spill:
                    continue
            raise RuntimeError("window packing failed; fallback not implemented")

    ecore = core
    ewin = awin[ecore, la]                # window of each edge
    ecol = acol[ecore, la]                # one-hot column of each edge
    gw = ecore * NWIN + ewin
    counts = np.bincount(gw, minlength=N_CORES * NWIN)
    assert counts.max() <= cap
    tt = NWIN * kt                        # tiles per core
    nslot = tt * 128                      # edge slots per core

    starts = np.zeros(N_CORES * NWIN, np.int64)
    np.cumsum(counts[:-1], out=starts[1:])
    # order edges by (core, window) to get slot ranks
    eorder = np.argsort(gw, kind="stable")
    rank = np.empty(si.shape[0], np.int64)
    rank[eorder] = np.arange(si.shape[0], dtype=np.int64) - starts[gw[eorder]]
    slot = ewin * (kt * 128) + rank       # slot within core

    # per-edge-slot positions; pads get r ~ 1e6 so every basis fn underflows to 0
    epos = np.zeros((N_CORES, nslot, 6), np.float32)
    epos[:, :, 3] = 1.0e6
    epos[core, slot, 0:3] = R[si]
    epos[core, slot, 3:6] = R[sj]

    tmat = np.zeros((N_CORES, nslot, 32), ml_dtypes.bfloat16)
    tmat[core, slot, ecol] = 1.0

    # DRAM layouts: partition dim = edge slot within tile (p = slot % 128)
    epos = np.ascontiguousarray(
        epos.reshape(N_CORES, tt, 128, 6).transpose(0, 2, 1, 3).reshape(N_CORES, 128, tt * 6)
    )
    tmat = np.ascontiguousarray(
        tmat.reshape(N_CORES, tt, 128, 32).transpose(0, 2, 1, 3).reshape(N_CORES, 128, tt * 32)
    )

    # --- weight constant-folding (symmetry reduction of the G1 contraction) ---
    # Feature order: [M0 (16)] [G1 j-major: r = 16 + j*16 + k1] [const-1]
    W1 = np.asarray(W1, np.float32)
    w1g = W1[16:].reshape(16, 16, HIDDEN)
    w1s = np.zeros((NFEAT, HIDDEN), np.float32)
    w1s[0:16] = W1[0:16]
    for j in range(9):
        for k1 in range(16):
            k2 = (k1 + j) % 16
            r = 16 + j * 16 + k1
            if j == 0:
                w1s[r] = w1g[k1, k1]
            elif j == 8:
                w1s[r] = 0.5 * (w1g[k1, k2] + w1g[k2, k1])
            else:
                w1s[r] = w1g[k1, k2] + w1g[k2, k1]
    w1s[160] = np.asarray(b1, np.float32)          # bias as a constant-1 feature

    W2 = np.asarray(W2, np.float32)
    W3 = np.asarray(W3, np.float32).reshape(HIDDEN)
    w1a = w1s[0:128].astype(np.float16)                               # [128, 512]
    w1b = w1s[128:161].astype(np.float16)                             # [33, 512]
    w2r = np.ascontiguousarray(
        W2.reshape(4, 128, HIDDEN).transpose(1, 0, 2).reshape(128, 4 * HIDDEN)
    )                                                                 # [128, 2048]
    w3r = np.ascontiguousarray(W3.reshape(4, 128).T)                  # [128, 4]
    b2t = np.ascontiguousarray(np.asarray(b2, np.float32).reshape(4, 128).T)
    b3s = np.asarray(b3, np.float32).reshape(1, 1)

    # per-atom scale / shift, [25, 512] per core, zero on pad atoms
    scale = np.asarray(scale, np.float32)
    shift = np.asarray(shift, np.float32)
    sflat = np.zeros((N_CORES, APC_PAD), np.float32)
    shflat = np.zeros((N_CORES, APC_PAD), np.float32)
    for c in range(N_CORES):
        zc = Z[c * APC:(c + 1) * APC]
        pos = awin[c] * WIN + acol[c]
        sflat[c, pos] = scale[zc]
        shflat[c, pos] = shift[zc]
    smat = np.zeros((N_CORES, EMAT_P, SC_BLK * 128), np.float32)
    shmat = np.zeros((N_CORES, EMAT_P, SC_BLK * 128), np.float32)
    b0 = 0
    for ch, nblk in enumerate(SC_BLOCKS):
        w = nblk * 128
        smat[:, ch, 0:w] = sflat[:, b0 * 128:b0 * 128 + w]
        shmat[:, ch, 0:w] = shflat[:, b0 * 128:b0 * 128 + w]
        b0 += nblk

    consts = dict(
        kt=kt, tt=tt,
        neg_beta=-1.0 / (2.0 * width * width),
        centers=[float(c) for c in centers],
    )
    per_core = dict(epos=epos, tmat=tmat, smat=smat, shmat=shmat)
    shared = dict(w1a=w1a, w1b=w1b, w2r=w2r, w3r=w3r, b2t=b2t, b3s=b3s)
    return consts, per_core, shared


# ------------------------------------------------------- numpy mirror (test)
def _mirror_core(c, consts, per_core, shared):
    """Pure-numpy emulation of the planned device program for one core, with
    the same dtype casts at the same places. Used by test.py for validation."""
    kt, tt = consts["kt"], consts["tt"]
    bf16 = ml_dtypes.bfloat16
    f32 = np.float32
    epos = per_core["epos"][c].reshape(128, tt, 6).transpose(1, 0, 2)  # [tt,128,6]
    tmat = per_core["tmat"][c].reshape(128, tt, 32).transpose(1, 0, 2)  # [tt,128,32]

    ri = epos[:, :, 0:3]
    rj = epos[:, :, 3:6]
    dr = rj - ri                                       # fp32
    r2 = (dr * dr).sum(-1)
    r = np.sqrt(r2)
    rinv = (1.0 / (r + EPS)).astype(f32)
    d = (dr * rinv[:, :, None]).astype(f32)            # [tt,128,3] f32
    cvec = np.asarray(consts["centers"], f32)
    diff = (r[:, :, None] - cvec[None, None, :]).astype(f32)
    bsq = (diff * diff).astype(f32)
    b = np.exp(consts["neg_beta"] * bsq).astype(bf16)  # [tt,128,16]
    # bd[c_,k] = b[k] * d[c_]  (c-major, matches feat_e[:, :, 16:64])
    bd = (b.astype(f32)[:, :, None, :] * d[:, :, :, None]).astype(bf16)  # [tt,128,3,16]
    feat_e = np.concatenate([b.reshape(tt, 128, 16),
                             bd.reshape(tt, 128, 48)], axis=-1)  # [tt,128,64] bf16

    # scatter: psum accumulate per window in fp32
    mfeat = np.zeros((APC_PAD, 64), f32)
    for w in range(NWIN):
        acc = np.zeros((32, 64), f32)
        for t in range(w * kt, (w + 1) * kt):
            acc += tmat[t].astype(f32).T @ feat_e[t].astype(f32)
        mfeat[w * 32:(w + 1) * 32] = acc

    m0 = mfeat[:, 0:16].astype(bf16)                    # [A,16]
    m1 = mfeat[:, 16:64].reshape(-1, 3, 16).astype(bf16)  # [A,3,16] (c,k)
    # m1e with wraparound cols 16:24 = m1[:, :, 0:8]
    m1e = np.concatenate([m1, m1[:, :, 0:8]], axis=-1)  # [A,3,24] bf16
    j = np.arange(9)[:, None]
    k1 = np.arange(16)[None, :]
    idx2 = (j + k1)                                     # [9,16] in 0..23
    a = m1e[:, :, 0:16].astype(f32)                     # in0 broadcast over j
    bb = m1e[:, :, :].astype(f32)[:, :, idx2]           # [A,3,9,16]
    p0 = (a[:, 0, None, :] * bb[:, 0]).astype(bf16)
    p1 = (a[:, 1, None, :] * bb[:, 1]).astype(bf16)
    s01 = (p0.astype(f32) + p1.astype(f32)).astype(bf16)
    p2 = (a[:, 2, None, :] * bb[:, 2]).astype(bf16)
    g1 = (s01.astype(f32) + p2.astype(f32)).astype(bf16)  # [A,9,16]
    feat = np.concatenate([m0, g1.reshape(-1, NPAIR),
                           np.ones((APC_PAD, 1), bf16)], axis=-1)  # [A,161] bf16
    fst = feat.astype(np.float16)                  # PSUM->SBUF copy cast

    # MLP: L1 in fp16 weights (psum f32 accumulate), L2/L3 modeled as fp32
    w1 = np.concatenate([shared["w1a"], shared["w1b"]], 0).astype(f32)
    b2v = shared["b2t"].T.reshape(HIDDEN)
    w2 = shared["w2r"].reshape(128, 4, HIDDEN).transpose(1, 0, 2).reshape(HIDDEN, HIDDEN).astype(f32)
    w3 = shared["w3r"].T.reshape(HIDDEN).astype(f32)

    h = fst.astype(f32) @ w1
    h = h / (1 + np.exp(-h))
    h = h @ w2 + b2v
    h = h / (1 + np.exp(-h))
    h3 = h @ w3 + float(shared["b3s"][0, 0])
    sm = np.zeros(APC_PAD, np.float32)
    sh = np.zeros(APC_PAD, np.float32)
    b0 = 0
    for ch, nblk in enumerate(SC_BLOCKS):
        w = nblk * 128
        sm[b0 * 128:b0 * 128 + w] = per_core["smat"][c][ch, 0:w]
        sh[b0 * 128:b0 * 128 + w] = per_core["shmat"][c][ch, 0:w]
        b0 += nblk
    return float(np.sum(sm * h3 + sh, dtype=np.float64))


def mirror(inputs):
    consts, per_core, shared = _preprocess(**inputs)
    return np.float32(sum(_mirror_core(c, consts, per_core, shared) for c in range(N_CORES)))


# ================================================================ device code
def _split_multi_waits(nc, mybir, max_waits=1):
    """Current walrus codegen rejects instructions carrying more than one
    semaphore wait. Hoist surplus waits onto NOPs inserted just before the
    instruction on the same (in-order) engine queue."""
    for f in nc.m.functions:
        for bb in f.blocks:
            out, changed = [], False
            for inst in bb.instructions:
                si = inst.sync_info
                waits = list(si.on_wait) if (si and si.on_wait) else []
                if len(waits) > max_waits:
                    extra, si.on_wait = waits[:-max_waits], waits[-max_waits:]
                    for k, w in enumerate(extra):
                        nop = mybir.InstNoOp(name=f"{inst.name}-wsplit{k}", ins=[], outs=[])
                        nop.engine = inst.engine
                        nop.sync_info = mybir.SyncInfo(on_wait=[w], on_update=[])
                        out.append(nop)
                    changed = True
                out.append(inst)
            if changed:
                bb.instructions = out


_PROG_CACHE = {}


def _get_program(kt, centers, neg_beta, b3val, num_devices=N_CORES, fix_waits=True):
    import concourse.bass as bass
    import concourse.mybir as mybir
    import concourse.tile as tile
    from concourse.tile import ScopedClock
    from concourse.masks import make_identity

    key = (kt, tuple(centers), neg_beta, b3val, num_devices, fix_waits)
    if key in _PROG_CACHE:
        return _PROG_CACHE[key]

    class SplitDrainTileContext(tile.TileContext):
        def _drain_and_barrier(self, tick_clock, wait_clock):
            drain_inst = self.nc.sync.drain()
            wait_clock.add_sem_waits(
                drain_inst.ins, ScopedClock({None: tick_clock.global_clock})
            )
            si = drain_inst.ins.sync_info
            waits = list(si.on_wait or [])
            if len(waits) > 1:
                si.on_wait = waits[:1]
                for w in waits[1:]:
                    extra = self.nc.sync.drain()
                    extra.ins.sync_info = mybir.SyncInfo(on_wait=[w], on_update=[])
            self.nc.all_engine_barrier()
            assert self.sems is not None
            popped = self.nc._tile_sem_poison_stack.pop()
            assert popped is self._sem_poison
            self.nc.clear_and_free_semaphores(list(self.sems.allocated().values()))
            self.nc.all_engine_barrier()
            _split_multi_waits(self.nc, mybir)

    TC = SplitDrainTileContext if fix_waits else tile.TileContext

    f32 = mybir.dt.float32
    f32r = mybir.dt.float32r
    bf = mybir.dt.bfloat16
    fp16 = mybir.dt.float16
    tt = NWIN * kt
    sc_blocks = list(SC_BLOCKS)

    nc = bass.Bass("TRN2", target_bir_lowering=False, debug=False, num_devices=num_devices)
    epos_d = nc.dram_tensor("epos", [128, tt * 6], f32, kind="ExternalInput")
    tmat_d = nc.dram_tensor("tmat", [128, tt * 32], bf, kind="ExternalInput")
    w1a_d = nc.dram_tensor("w1a", [128, HIDDEN], fp16, kind="ExternalInput")
    w1b_d = nc.dram_tensor("w1b", [33, HIDDEN], fp16, kind="ExternalInput")
    w2r_d = nc.dram_tensor("w2r", [128, 4 * HIDDEN], f32r, kind="ExternalInput")
    w3r_d = nc.dram_tensor("w3r", [128, 4], f32r, kind="ExternalInput")
    b2t_d = nc.dram_tensor("b2t", [128, 4], f32, kind="ExternalInput")
    b3s_d = nc.dram_tensor("b3s", [1, 1], f32, kind="ExternalInput")
    smat_d = nc.dram_tensor("smat", [EMAT_P, SC_BLK * 128], f32, kind="ExternalInput")
    shm_d = nc.dram_tensor("shm", [EMAT_P, SC_BLK * 128], f32, kind="ExternalInput")
    eout_d = nc.dram_tensor("eout", [1, 1], f32, kind="ExternalOutput")

    with TC(nc) as tc:
        with tc.tile_pool(name="const", bufs=1) as cpool:
            ident = cpool.tile([128, 128], f32)
            make_identity(nc, ident[:])
            ident_b = cpool.tile([128, 128], bf)
            nc.vector.tensor_copy(out=ident_b[:], in_=ident[:])
            identb = ident_b[:]
            negc = cpool.tile([128, 16], f32)
            for k in range(16):
                nc.vector.memset(negc[:, k:k + 1], float(-centers[k]))
            w1a_t = cpool.tile([128, HIDDEN], fp16)
            w1b_t = cpool.tile([33, HIDDEN], fp16)
            w2r_t = cpool.tile([128, 4 * HIDDEN], f32r)
            w3r_t = cpool.tile([128, 4], f32r)
            b2t_t = cpool.tile([128, 4], f32)
            b3s_t = cpool.tile([1, 1], f32)
            smat_t = cpool.tile([EMAT_P, SC_BLK * 128], f32)
            shm_t = cpool.tile([EMAT_P, SC_BLK * 128], f32)
            emat_t = cpool.tile([EMAT_P, SC_BLK * 128], f32)
            for dst, src in [(w1a_t, w1a_d), (w1b_t, w1b_d), (w2r_t, w2r_d),
                             (w3r_t, w3r_d), (b2t_t, b2t_d),
                             (b3s_t, b3s_d), (smat_t, smat_d), (shm_t, shm_d)]:
                nc.sync.dma_start(out=dst[:], in_=src.ap())
            nc.vector.memset(emat_t[:], 0.0)

            # ------------- fused pipeline over superchunks (4 blocks = 512 atoms)
            # The edge chain (dr -> r -> basis -> b*d) has ~11us of serial
            # latency woven across DVE/Act/Pool, so it is issued two
            # superchunks ahead of the scatter+MLP stage that consumes it
            # (eio/emid bufs=3 hold the in-flight superchunks).
            with tc.tile_pool(name="eio", bufs=3) as eio, \
                 tc.tile_pool(name="emid", bufs=3) as emid, \
                 tc.tile_pool(name="msc", bufs=2) as msc, \
                 tc.tile_pool(name="hmid", bufs=2) as hmid, \
                 tc.tile_pool(name="bio", bufs=2) as bio, \
                 tc.tile_pool(name="psA", bufs=2, space="PSUM") as psA_p, \
                 tc.tile_pool(name="psT", bufs=1, space="PSUM") as psT_p, \
                 tc.tile_pool(name="psB", bufs=3, space="PSUM") as psB_p:

                def emit_edge(ch, nblk, blk0):
                    nt = nblk * 4 * kt             # edge tiles
                    t0 = blk0 * 4 * kt

                    epos_t = emid.tile([128, nt, 6], f32, tag="epos")
                    tmat_t = eio.tile([128, nt, 32], bf, tag="tmat")
                    nc.sync.dma_start(out=epos_t[:].rearrange("p a b -> p (a b)"),
                                      in_=epos_d.ap()[:, t0 * 6:(t0 + nt) * 6])
                    nc.sync.dma_start(out=tmat_t[:].rearrange("p a b -> p (a b)"),
                                      in_=tmat_d.ap()[:, t0 * 32:(t0 + nt) * 32])

                    drt = emid.tile([128, nt, 3], f32, tag="drt")
                    sqt = emid.tile([128, nt, 3], f32, tag="sqt")
                    r2t = emid.tile([128, nt], f32, tag="r2t")
                    rt = emid.tile([128, nt], f32, tag="rt")
                    rit = emid.tile([128, nt], f32, tag="rit")
                    diff = emid.tile([128, nt, 16], f32, tag="diff", bufs=2)
                    feat_e = eio.tile([128, nt, 64], bf, tag="feate")

                    nc.vector.tensor_tensor(out=drt[:], in0=epos_t[:, :, 3:6],
                                            in1=epos_t[:, :, 0:3], op=mybir.AluOpType.subtract)
                    nc.vector.tensor_tensor(out=sqt[:], in0=drt[:], in1=drt[:],
                                            op=mybir.AluOpType.mult)
                    nc.vector.tensor_reduce(out=r2t[:], in_=sqt[:],
                                            axis=mybir.AxisListType.X, op=mybir.AluOpType.add)
                    nc.scalar.activation(out=rt[:], in_=r2t[:],
                                         func=mybir.ActivationFunctionType.Sqrt)
                    nc.vector.tensor_scalar(out=r2t[:], in0=rt[:], scalar1=EPS, scalar2=None,
                                            op0=mybir.AluOpType.add)
                    nc.vector.reciprocal(out=rit[:], in_=r2t[:])
                    nc.vector.tensor_tensor(out=drt[:], in0=drt[:],
                                            in1=rit[:].unsqueeze(2).broadcast_to([128, nt, 3]),
                                            op=mybir.AluOpType.mult)
                    # diff[p,t,k] = r - c_k; square in place on Pool
                    nc.gpsimd.tensor_tensor(
                        out=diff[:],
                        in0=rt[:].unsqueeze(2).broadcast_to([128, nt, 16]),
                        in1=negc[:].unsqueeze(1).broadcast_to([128, nt, 16]),
                        op=mybir.AluOpType.add)
                    nc.gpsimd.tensor_tensor(out=diff[:], in0=diff[:], in1=diff[:],
                                            op=mybir.AluOpType.mult)
                    nc.scalar.activation(out=feat_e[:, :, 0:16], in_=diff[:],
                                         func=mybir.ActivationFunctionType.Exp,
                                         scale=float(neg_beta))
                    bdout = feat_e[:, :, 16:64].rearrange("p t (c k) -> p t c k", c=3)
                    bview = feat_e[:, :, 0:16].unsqueeze(2).broadcast_to([128, nt, 3, 16])
                    nc.vector.tensor_tensor(
                        out=bdout[:],
                        in0=bview,
                        in1=drt[:].unsqueeze(3).broadcast_to([128, nt, 3, 16]),
                        op=mybir.AluOpType.mult)
                    return tmat_t, feat_e

                def emit_scatter(ch, nblk, blk0, tmat_t, feat_e):
                    nt = nblk * 4 * kt
                    w = nblk * 128
                    # scatter + moment extraction (batched per superchunk)
                    m1e = msc.tile([128, nblk, 3, 24], bf, tag="m1e")
                    feat_at = msc.tile([128, nblk, NFEAT], bf, tag="feat_at")
                    nc.vector.memset(feat_at[:, :, 160:161], 1.0)
                    psA = psA_p.tile([128, nblk, 64], f32, tag="psA")
                    for b in range(nblk):
                        for wv in range(4):
                            for k in range(kt):
                                ti = (b * 4 + wv) * kt + k
                                nc.tensor.matmul(
                                    out=psA[32 * wv:32 * wv + 32, b, :],
                                    lhsT=tmat_t[:, ti, :], rhs=feat_e[:, ti, :],
                                    start=(k == 0), stop=(k == kt - 1),
                                    tile_position=(0, 32 * wv), skip_group_check=True)
                    nc.vector.tensor_copy(out=feat_at[:, :, 0:16], in_=psA[:, :, 0:16])
                    psA_ck = psA[:, :, 16:64].rearrange("p b (c k) -> p b c k", c=3)
                    nc.vector.tensor_copy(out=m1e[:, :, :, 0:16], in_=psA_ck)
                    nc.vector.tensor_copy(out=m1e[:, :, :, 16:24], in_=psA_ck[:, :, :, 0:8])

                    # j-outer symmetric contraction: G1[j,k1] = sum_d m1[d,k1]*m1[d,k1+j]
                    g1a = msc.tile([128, nblk, 9, 16], bf, tag="g1a")
                    g1b = msc.tile([128, nblk, 9, 16], bf, tag="g1b")
                    g1c = msc.tile([128, nblk, 9, 16], bf, tag="g1c")
                    for d, dst in ((0, g1a), (1, g1b), (2, g1c)):
                        nc.vector.tensor_tensor(
                            out=dst[:],
                            in0=m1e[:, :, d, 0:16].unsqueeze(2).broadcast_to([128, nblk, 9, 16]),
                            in1=_diag_view_j(m1e, d, nblk),
                            op=mybir.AluOpType.mult)
                    nc.vector.tensor_tensor(out=g1a[:], in0=g1a[:], in1=g1b[:],
                                            op=mybir.AluOpType.add)
                    nc.vector.tensor_tensor(
                        out=feat_at[:, :, 16:160].rearrange("p b (j k) -> p b j k", j=9),
                        in0=g1a[:], in1=g1c[:], op=mybir.AluOpType.add)
                    return feat_at

                def emit_mlp(ch, nblk, blk0, feat_at):
                    w = nblk * 128
                    # transpose features to [feat, atoms]
                    psT0 = psT_p.tile([128, nblk, 128], bf, tag="psT0")
                    psT1 = psT_p.tile([33, nblk, 128], bf, tag="psT1")
                    for b in range(nblk):
                        nc.tensor.matmul(out=psT0[:, b, :], lhsT=feat_at[:, b, 0:128],
                                         rhs=identb, is_transpose=True,
                                         start=True, stop=True, skip_group_check=True)
                        nc.tensor.matmul(out=psT1[:, b, :], lhsT=feat_at[:, b, 128:161],
                                         rhs=identb, is_transpose=True,
                                         start=True, stop=True, skip_group_check=True)
                    fst0 = msc.tile([128, nblk, 128], fp16, tag="fst0")
                    fst1 = msc.tile([33, nblk, 128], fp16, tag="fst1")
                    nc.vector.tensor_copy(out=fst0[:], in_=psT0[:])
                    nc.vector.tensor_copy(out=fst1[:], in_=psT1[:])

                    # MLP in 512-atom sub-chunks
                    for s0 in range(0, nblk, 4):
                        sb = min(4, nblk - s0)
                        ws = sb * 128
                        f0c = fst0[:, s0:s0 + sb, :].rearrange("p b x -> p (b x)")
                        f1c = fst1[:, s0:s0 + sb, :].rearrange("p b x -> p (b x)")
                        h1s = hmid.tile([128, 4, ws], f32r, tag="h1s")
                        for h in range(4):
                            ps = psB_p.tile([128, ws], f32, tag="psB")
                            nc.tensor.matmul(out=ps[:], lhsT=w1a_t[:, h * 128:(h + 1) * 128],
                                             rhs=f0c, start=True, stop=False, skip_group_check=True)
                            nc.tensor.matmul(out=ps[:], lhsT=w1b_t[:, h * 128:(h + 1) * 128],
                                             rhs=f1c, start=False, stop=True, skip_group_check=True)
                            nc.scalar.activation(out=h1s[:, h, :], in_=ps[:],
                                                 func=mybir.ActivationFunctionType.Silu)
                        h2s = hmid.tile([128, 4, ws], f32r, tag="h2s")
                        for h in range(4):
                            ps = psB_p.tile([128, ws], f32, tag="psB")
                            for k in range(4):
                                nc.tensor.matmul(
                                    out=ps[:], lhsT=w2r_t[:, k * HIDDEN + h * 128:k * HIDDEN + (h + 1) * 128],
                                    rhs=h1s[:, k, :], start=(k == 0), stop=(k == 3), skip_group_check=True)
                            nc.scalar.activation(out=h2s[:, h, :], in_=ps[:],
                                                 func=mybir.ActivationFunctionType.Silu,
                                                 bias=b2t_t[:, h:h + 1])
                        pse = psB_p.tile([128, ws], f32, tag="psB")
                        for k in range(4):
                            nc.tensor.matmul(out=pse[0:1, :], lhsT=w3r_t[:, k:k + 1],
                                             rhs=h2s[:, k, :], start=(k == 0), stop=(k == 3), skip_group_check=True)
                        erow = bio.tile([1, ws], f32, tag="erow")
                        nc.scalar.activation(out=erow[:], in_=pse[0:1, :],
                                             func=mybir.ActivationFunctionType.Identity,
                                             bias=b3s_t[:])
                        nc.sync.dma_start(out=emat_t[ch:ch + 1, s0 * 128:s0 * 128 + ws],
                                          in_=erow[:])

                # driver: three-stage software pipeline. Edge chains run two
                # superchunks ahead of their scatter; transposes+MLP trail the
                # scatter by one superchunk so the g1 contraction has a full
                # period of slack before the transposes consume it.
                blk0s = []
                b0 = 0
                for nblk in sc_blocks:
                    blk0s.append(b0)
                    b0 += nblk
                n_sc = len(sc_blocks)
                edges = {}
                feats = {}

                def do_edge(u):
                    edges[u] = emit_edge(u, sc_blocks[u], blk0s[u])

                def do_scatter(u):
                    tm, fe = edges.pop(u)
                    feats[u] = emit_scatter(u, sc_blocks[u], blk0s[u], tm, fe)

                def do_mlp(u):
                    emit_mlp(u, sc_blocks[u], blk0s[u], feats.pop(u))

                for u in range(3):
                    do_edge(u)
                do_scatter(0)
                for u in range(1, n_sc):
                    do_mlp(u - 1)
                    if u + 2 < n_sc:
                        do_edge(u + 2)
                    do_scatter(u)
                do_mlp(n_sc - 1)

                # final readout
                u = bio.tile([EMAT_P, SC_BLK * 128], f32, tag="u", bufs=1)
                acc = bio.tile([EMAT_P, 1], f32, tag="acc", bufs=1)
                nc.vector.tensor_tensor(out=u[:], in0=emat_t[:], in1=smat_t[:],
                                        op=mybir.AluOpType.mult)
                nc.vector.scalar_tensor_tensor(out=u[:], in0=u[:], scalar=1.0, in1=shm_t[:],
                                               op0=mybir.AluOpType.mult,
                                               op1=mybir.AluOpType.add, accum_out=acc[:])
                ones = bio.tile([EMAT_P, 1], f32, tag="ones", bufs=1)
                nc.vector.memset(ones[:], 1.0)
                psf = psB_p.tile([128, 1], f32, tag="psB")
                nc.tensor.matmul(out=psf[0:1, :], lhsT=ones[:], rhs=acc[:], start=True,
                                 stop=True, skip_group_check=True)
                eo = bio.tile([1, 1], f32, tag="eo", bufs=1)
                nc.scalar.activation(out=eo[:], in_=psf[0:1, :],
                                     func=mybir.ActivationFunctionType.Copy)
                nc.sync.dma_start(out=eout_d.ap(), in_=eo[:])

    _PROG_CACHE[key] = nc
    return nc


def _diag_view_j1(m1e, b, d):
    """AP [128, j(9), k1(16)] reading m1e[:, b, d, j + k1] (both dims stride 1
    -> packed operand, eligible for the DVE 2x mode)."""
    import concourse.ap as cap

    base = m1e[:, b, d, :]                         # [128, 24]
    v = base.unsqueeze(1)                          # [128, 1, 24]
    v = v[:, :, 0:16]                              # [128, 1, 16]
    v = v.broadcast_to([128, 9, 16])               # j dim stride 0
    apl = [list(p) for p in v.ap]
    apl[-2] = [1, 9]
    return cap.AP(v.tensor, v.offset, apl, v.const_val, v.runtime_checks,
                  v.dep_tracking_offset)


def _diag_view_jr(m1e, a, b, d):
    """AP [128, b-a, j(9), k1(16)] reading m1e[:, a:b, d, j + k1]."""
    import concourse.ap as cap

    base = m1e[:, a:b, d, :]
    v = base.unsqueeze(2)
    v = v[:, :, :, 0:16]
    v = v.broadcast_to([128, b - a, 9, 16])
    apl = [list(p) for p in v.ap]
    apl[-2] = [1, 9]
    return cap.AP(v.tensor, v.offset, apl, v.const_val, v.runtime_checks,
                  v.dep_tracking_offset)


def _diag_view_j(m1e, d, nblk):
    """AP [128, nblk, j(9), k1(16)] reading m1e[:, :, d, j + k1] (both trailing
    dims stride 1 -> packed operand, eligible for the DVE 2x mode)."""
    import concourse.ap as cap

    base = m1e[:, :, d, :]                         # [128, nblk, 24]
    v = base.unsqueeze(2)                          # [128, nblk, 1, 24]
    v = v[:, :, :, 0:16]                           # [128, nblk, 1, 16]
    v = v.broadcast_to([128, nblk, 9, 16])         # j dim stride 0
    apl = [list(p) for p in v.ap]
    apl[-2] = [1, 9]
    return cap.AP(v.tensor, v.offset, apl, v.const_val, v.runtime_checks,
                  v.dep_tracking_offset)


LAST_EXEC_NS = None
PROFILE = False


def kernel(**inputs):
    from concourse.bass_utils import run_bass_kernel_spmd

    consts, per_core, shared = _preprocess(**inputs)
    nc = _get_program(consts["kt"], consts["centers"], consts["neg_beta"],
                      float(np.asarray(shared["b3s"]).reshape(())))
    in_maps = []
    for c in range(N_CORES):
        m = dict(
            epos=per_core["epos"][c],
            tmat=per_core["tmat"][c],
            w1a=shared["w1a"], w1b=shared["w1b"],
            w2r=shared["w2r"], w3r=shared["w3r"],
            b2t=shared["b2t"], b3s=shared["b3s"],
            smat=per_core["smat"][c], shm=per_core["shmat"][c],
        )
        in_maps.append(m)
    global LAST_EXEC_NS
    kwargs = {}
    if PROFILE:
        import tempfile
        kwargs = dict(trace=True, tmpdir=tempfile.mkdtemp(prefix="ktrace_"))
    res = run_bass_kernel_spmd(nc, in_maps, core_ids=list(range(N_CORES)), **kwargs)
    if getattr(res, "exec_time_ns", None):
        LAST_EXEC_NS = res.exec_time_ns
    if PROFILE:
        globals()["LAST_RESULTS"] = res
    total = np.float32(0.0)
    for c in range(N_CORES):
        total += np.float32(res.results[c]["eout"].reshape(()))
    return np.float32(total)


# revision 46
# speedup vs baseline: 4.5613x; 4.5613x over previous
"""Trainium2 Bass kernel for a gaussian-moment energy model (GNN message passing).

Strategy (8 NeuronCores, SPMD single program):
  - Host: partition atoms into 8 contiguous shards of 12500 (padded to 13056
    = 102 blocks of 128). Edges owned by the receiver atom i. Within each
    core, atoms are BIN-PACKED by degree into 408 windows of <=32 atoms so
    every window's edge list fits in exactly KT_PACK=4 tiles of 128 slots
    (~2% slot padding instead of ~25% with contiguous windows). Pre-gather
    R[i]/R[j] per edge slot (the "halo gather" of the sharding hint) and the
    per-edge one-hot window-assignment matrix T (exact 0/1 in bf16).
  - Device per core, single fused pipeline over superchunks of 8 atom-blocks
    (1024 atoms), with the edge chain software-pipelined two superchunks
    ahead of the scatter/MLP stage: edge pipeline (dr, r, unit dir, 16
    gaussian basis fns via one broadcast-subtract on GPSIMD + Square + Exp,
    b*d features; spread over DVE/Act/GPSIMD) -> one-hot scatter matmuls
    into PSUM quadrants (per-atom M0/M1 moments) -> j-outer symmetric
    gaussian-moment contraction in bf16 (both operands packed-innermost =
    2x DVE mode; W1 is re-folded on the host to match the j-major feature
    order) -> PE transpose (bf16) -> 3-layer silu MLP (layer-1 in fp16 with
    its bias folded in as a constant-1 feature, layers 2/3 in f32r) ->
    per-atom scale/shift -> scalar sum. Features stay in SBUF end to end.
  - Host: sum the 8 per-core partial energies.
"""

import math
import numpy as np
import ml_dtypes

# ---------------------------------------------------------------- constants
N_ATOMS = 100_000
N_EDGES = 1_600_000
N_RADIAL = 16
HIDDEN = 512
EPS = 1e-8

N_CORES = 8
APC = 12_500          # real atoms per core
# Atoms are bin-packed into windows by degree so every 32-atom window fits in
# KT_PACK*128 edge slots; the extra atom padding buys the packing slack.
APC_PAD = 13_056      # 102 blocks of 128
WIN = 32              # atoms per scatter window (psum col-group)
NWIN = APC_PAD // WIN  # 408
NBLK = APC_PAD // 128  # 102
KT_PACK = 4           # tiles per window when packing succeeds (bf16 path)
KT_BLK = 16           # tiles per 128-atom block (fp8 DoubleRow path)
NPAIR = 144           # 9 j x 16 k1 symmetric-pair slots (j-major)
NFEAT = 16 + NPAIR + 1  # 161: M0, G1, constant-1 (carries the layer-1 bias)
SC_BLK = 8            # max atom blocks per superchunk
SC_BLOCKS = [8] * 12 + [6]
FP8_SCATTER = False   # fp8e4m3 + block-level DoubleRow scatter: compiles, but
                      # the exec unit dies with NRT_EXEC_UNIT_UNRECOVERABLE on
                      # real TRN2 — do not enable.
assert sum(SC_BLOCKS) == NBLK
EMAT_P = len(SC_BLOCKS)  # rows in the readout matrix (one per superchunk)


def _preprocess(R, Z, idx, centers, width, W1, b1, W2, b2, W3, b3, scale, shift,
                box=None, offsets=None):
    """Host-side graph partitioning / sharding. Index manipulation + gathers
    only (plus compile-time constant folding of the MLP weights)."""
    R = np.asarray(R, np.float32)
    Z = np.asarray(Z)
    idx_i = np.asarray(idx[0]).astype(np.int64)
    idx_j = np.asarray(idx[1]).astype(np.int64)
    centers = np.asarray(centers, np.float32)
    width = float(np.asarray(width))

    order = np.argsort(idx_i, kind="stable")
    si = idx_i[order]
    sj = idx_j[order]

    core = si // APC                      # owner core per edge
    la = si - core * APC                  # local atom id [0, APC)

    # --- degree-balanced bin packing of atoms into windows (per core) ---
    # Worst-fit decreasing on edge count, respecting <=WIN atoms and
    # <=KT_PACK*128 edges per window. On success every window needs only
    # KT_PACK tiles; on failure fall back to more tiles per window.
    import heapq
    if FP8_SCATTER:
        nbins, bwidth, kt = NBLK, 128, KT_BLK
    else:
        nbins, bwidth, kt = NWIN, WIN, KT_PACK
    cap = kt * 128
    # position of each local atom: bin id and column within bin
    awin = np.zeros((N_CORES, APC), np.int32)
    acol = np.zeros((N_CORES, APC), np.int32)
    for c in range(N_CORES):
        deg = np.bincount(la[core == c], minlength=APC)
        order_a = np.argsort(-deg, kind="stable")
        heap = [(0, 0, w) for w in range(nbins)]  # (edges_used, atoms_used, w)
        ok = True
        spill = []
        for a in order_a:
            d = int(deg[a])
            e_used, n_used, wdx = heap[0]
            if e_used + d > cap or n_used >= bwidth:
                # try a few more candidates before declaring failure
                tmp = []
                placed = False
                for _ in range(min(64, len(heap))):
                    e2, n2, w2 = heapq.heappop(heap)
                    if e2 + d <= cap and n2 < bwidth:
                        awin[c, a] = w2
                        acol[c, a] = n2
                        heapq.heappush(heap, (e2 + d, n2 + 1, w2))
                        placed = True
                        break
                    tmp.append((e2, n2, w2))
                for t in tmp:
                    heapq.heappush(heap, t)
                if not placed:
                    ok = False
                    spill.append(a)
                continue
            heapq.heappop(heap)
            awin[c, a] = wdx
            acol[c, a] = n_used
            heapq.heappush(heap, (e_used + d, n_used + 1, wdx))
        if not ok:
            raise RuntimeError("window packing failed; fallback not implemented")

    ecore = core
    ewin = awin[ecore, la]                # bin of each edge
    ecol = acol[ecore, la]                # one-hot column of each edge
    gw = ecore * nbins + ewin
    counts = np.bincount(gw, minlength=N_CORES * nbins)
    assert counts.max() <= cap
    tt = nbins * kt                       # tiles per core
    nslot = tt * 128                      # edge slots per core

    starts = np.zeros(N_CORES * nbins, np.int64)
    np.cumsum(counts[:-1], out=starts[1:])
    # order edges by (core, window) to get slot ranks
    eorder = np.argsort(gw, kind="stable")
    rank = np.empty(si.shape[0], np.int64)
    rank[eorder] = np.arange(si.shape[0], dtype=np.int64) - starts[gw[eorder]]
    slot = ewin * (kt * 128) + rank       # slot within core

    # per-edge-slot positions; pads get r ~ 1e6 so every basis fn underflows to 0
    epos = np.zeros((N_CORES, nslot, 6), np.float32)
    epos[:, :, 3] = 1.0e6
    epos[core, slot, 0:3] = R[si]
    epos[core, slot, 3:6] = R[sj]

    tdt = ml_dtypes.float8_e4m3 if FP8_SCATTER else ml_dtypes.bfloat16
    twid = 128 if FP8_SCATTER else 32
    tmat = np.zeros((N_CORES, nslot, twid), tdt)
    tmat[core, slot, ecol] = 1.0

    # DRAM layouts: partition dim = edge slot within tile (p = slot % 128)
    epos = np.ascontiguousarray(
        epos.reshape(N_CORES, tt, 128, 6).transpose(0, 2, 1, 3).reshape(N_CORES, 128, tt * 6)
    )
    tmat = np.ascontiguousarray(
        tmat.reshape(N_CORES, tt, 128, twid).transpose(0, 2, 1, 3)
        .reshape(N_CORES, 128, tt * twid)
    )

    # --- weight constant-folding (symmetry reduction of the G1 contraction) ---
    # Feature order: [M0 (16)] [G1 j-major: r = 16 + j*16 + k1] [const-1]
    W1 = np.asarray(W1, np.float32)
    w1g = W1[16:].reshape(16, 16, HIDDEN)
    w1s = np.zeros((NFEAT, HIDDEN), np.float32)
    w1s[0:16] = W1[0:16]
    for j in range(9):
        for k1 in range(16):
            k2 = (k1 + j) % 16
            r = 16 + j * 16 + k1
            if j == 0:
                w1s[r] = w1g[k1, k1]
            elif j == 8:
                w1s[r] = 0.5 * (w1g[k1, k2] + w1g[k2, k1])
            else:
                w1s[r] = w1g[k1, k2] + w1g[k2, k1]
    w1s[160] = np.asarray(b1, np.float32)          # bias as a constant-1 feature

    W2 = np.asarray(W2, np.float32)
    W3 = np.asarray(W3, np.float32).reshape(HIDDEN)
    w1a = w1s[0:128].astype(np.float16)                               # [128, 512]
    w1b = w1s[128:161].astype(np.float16)                             # [33, 512]
    w2r = np.ascontiguousarray(
        W2.reshape(4, 128, HIDDEN).transpose(1, 0, 2).reshape(128, 4 * HIDDEN)
    )                                                                 # [128, 2048]
    w3r = np.ascontiguousarray(W3.reshape(4, 128).T)                  # [128, 4]
    b2t = np.ascontiguousarray(np.asarray(b2, np.float32).reshape(4, 128).T)
    b3s = np.asarray(b3, np.float32).reshape(1, 1)

    # per-atom scale / shift, [25, 512] per core, zero on pad atoms
    scale = np.asarray(scale, np.float32)
    shift = np.asarray(shift, np.float32)
    sflat = np.zeros((N_CORES, APC_PAD), np.float32)
    shflat = np.zeros((N_CORES, APC_PAD), np.float32)
    for c in range(N_CORES):
        zc = Z[c * APC:(c + 1) * APC]
        pos = awin[c] * (128 if FP8_SCATTER else WIN) + acol[c]
        sflat[c, pos] = scale[zc]
        shflat[c, pos] = shift[zc]
    smat = np.zeros((N_CORES, EMAT_P, SC_BLK * 128), np.float32)
    shmat = np.zeros((N_CORES, EMAT_P, SC_BLK * 128), np.float32)
    b0 = 0
    for ch, nblk in enumerate(SC_BLOCKS):
        w = nblk * 128
        smat[:, ch, 0:w] = sflat[:, b0 * 128:b0 * 128 + w]
        shmat[:, ch, 0:w] = shflat[:, b0 * 128:b0 * 128 + w]
        b0 += nblk

    consts = dict(
        kt=kt, tt=tt,
        neg_beta=-1.0 / (2.0 * width * width),
        centers=[float(c) for c in centers],
    )
    per_core = dict(epos=epos, tmat=tmat, smat=smat, shmat=shmat)
    shared = dict(w1a=w1a, w1b=w1b, w2r=w2r, w3r=w3r, b2t=b2t, b3s=b3s)
    return consts, per_core, shared


# ------------------------------------------------------- numpy mirror (test)
def _mirror_core(c, consts, per_core, shared):
    """Pure-numpy emulation of the planned device program for one core, with
    the same dtype casts at the same places. Used by test.py for validation."""
    kt, tt = consts["kt"], consts["tt"]
    bf16 = ml_dtypes.bfloat16
    edt = ml_dtypes.float8_e4m3 if FP8_SCATTER else bf16
    f32 = np.float32
    epos = per_core["epos"][c].reshape(128, tt, 6).transpose(1, 0, 2)  # [tt,128,6]
    twid = 128 if FP8_SCATTER else 32
    tmat = per_core["tmat"][c].reshape(128, tt, twid).transpose(1, 0, 2)

    ri = epos[:, :, 0:3]
    rj = epos[:, :, 3:6]
    dr = rj - ri                                       # fp32
    r2 = (dr * dr).sum(-1)
    r = np.sqrt(r2)
    rinv = (1.0 / (r + EPS)).astype(f32)
    d = (dr * rinv[:, :, None]).astype(f32)            # [tt,128,3] f32
    cvec = np.asarray(consts["centers"], f32)
    diff = (r[:, :, None] - cvec[None, None, :]).astype(f32)
    bsq = (diff * diff).astype(f32)
    b = np.exp(consts["neg_beta"] * bsq).astype(edt)   # [tt,128,16]
    # bd[c_,k] = b[k] * d[c_]  (c-major, matches feat_e[:, :, 16:64])
    bd = (b.astype(f32)[:, :, None, :] * d[:, :, :, None]).astype(edt)  # [tt,128,3,16]
    feat_e = np.concatenate([b.reshape(tt, 128, 16),
                             bd.reshape(tt, 128, 48)], axis=-1)  # [tt,128,64] bf16

    # scatter: psum accumulate per bin in fp32
    mfeat = np.zeros((APC_PAD, 64), f32)
    bwidth = 128 if FP8_SCATTER else WIN
    nbins = APC_PAD // bwidth
    for w in range(nbins):
        acc = np.zeros((bwidth, 64), f32)
        for t in range(w * kt, (w + 1) * kt):
            acc += tmat[t].astype(f32).T @ feat_e[t].astype(f32)
        mfeat[w * bwidth:(w + 1) * bwidth] = acc

    m0 = mfeat[:, 0:16].astype(bf16)                    # [A,16]
    m1 = mfeat[:, 16:64].reshape(-1, 3, 16).astype(bf16)  # [A,3,16] (c,k)
    # m1e with wraparound cols 16:24 = m1[:, :, 0:8]
    m1e = np.concatenate([m1, m1[:, :, 0:8]], axis=-1)  # [A,3,24] bf16
    j = np.arange(9)[:, None]
    k1 = np.arange(16)[None, :]
    idx2 = (j + k1)                                     # [9,16] in 0..23
    a = m1e[:, :, 0:16].astype(f32)                     # in0 broadcast over j
    bb = m1e[:, :, :].astype(f32)[:, :, idx2]           # [A,3,9,16]
    p0 = (a[:, 0, None, :] * bb[:, 0]).astype(bf16)
    p1 = (a[:, 1, None, :] * bb[:, 1]).astype(bf16)
    s01 = (p0.astype(f32) + p1.astype(f32)).astype(bf16)
    p2 = (a[:, 2, None, :] * bb[:, 2]).astype(bf16)
    g1 = (s01.astype(f32) + p2.astype(f32)).astype(bf16)  # [A,9,16]
    feat = np.concatenate([m0, g1.reshape(-1, NPAIR),
                           np.ones((APC_PAD, 1), bf16)], axis=-1)  # [A,161] bf16
    fst = feat.astype(np.float16)                  # PSUM->SBUF copy cast

    # MLP: L1 in fp16 weights (psum f32 accumulate), L2/L3 modeled as fp32
    w1 = np.concatenate([shared["w1a"], shared["w1b"]], 0).astype(f32)
    b2v = shared["b2t"].T.reshape(HIDDEN)
    w2 = shared["w2r"].reshape(128, 4, HIDDEN).transpose(1, 0, 2).reshape(HIDDEN, HIDDEN).astype(f32)
    w3 = shared["w3r"].T.reshape(HIDDEN).astype(f32)

    h = fst.astype(f32) @ w1
    h = h / (1 + np.exp(-h))
    h = h @ w2 + b2v
    h = h / (1 + np.exp(-h))
    h3 = h @ w3 + float(shared["b3s"][0, 0])
    sm = np.zeros(APC_PAD, np.float32)
    sh = np.zeros(APC_PAD, np.float32)
    b0 = 0
    for ch, nblk in enumerate(SC_BLOCKS):
        w = nblk * 128
        sm[b0 * 128:b0 * 128 + w] = per_core["smat"][c][ch, 0:w]
        sh[b0 * 128:b0 * 128 + w] = per_core["shmat"][c][ch, 0:w]
        b0 += nblk
    return float(np.sum(sm * h3 + sh, dtype=np.float64))


def mirror(inputs):
    consts, per_core, shared = _preprocess(**inputs)
    return np.float32(sum(_mirror_core(c, consts, per_core, shared) for c in range(N_CORES)))


# ================================================================ device code
def _split_multi_waits(nc, mybir, max_waits=1):
    """Current walrus codegen rejects instructions carrying more than one
    semaphore wait. Hoist surplus waits onto NOPs inserted just before the
    instruction on the same (in-order) engine queue."""
    for f in nc.m.functions:
        for bb in f.blocks:
            out, changed = [], False
            for inst in bb.instructions:
                si = inst.sync_info
                waits = list(si.on_wait) if (si and si.on_wait) else []
                if len(waits) > max_waits:
                    extra, si.on_wait = waits[:-max_waits], waits[-max_waits:]
                    for k, w in enumerate(extra):
                        nop = mybir.InstNoOp(name=f"{inst.name}-wsplit{k}", ins=[], outs=[])
                        nop.engine = inst.engine
                        nop.sync_info = mybir.SyncInfo(on_wait=[w], on_update=[])
                        out.append(nop)
                    changed = True
                out.append(inst)
            if changed:
                bb.instructions = out


_PROG_CACHE = {}


def _get_program(kt, centers, neg_beta, b3val, num_devices=N_CORES, fix_waits=True,
                 reps=1):
    import concourse.bass as bass
    import concourse.mybir as mybir
    import concourse.tile as tile
    from concourse.tile import ScopedClock
    from concourse.masks import make_identity

    key = (kt, tuple(centers), neg_beta, b3val, num_devices, fix_waits, reps,
           FP8_SCATTER)
    if key in _PROG_CACHE:
        return _PROG_CACHE[key]

    class SplitDrainTileContext(tile.TileContext):
        def _drain_and_barrier(self, tick_clock, wait_clock):
            drain_inst = self.nc.sync.drain()
            wait_clock.add_sem_waits(
                drain_inst.ins, ScopedClock({None: tick_clock.global_clock})
            )
            si = drain_inst.ins.sync_info
            waits = list(si.on_wait or [])
            if len(waits) > 1:
                si.on_wait = waits[:1]
                for w in waits[1:]:
                    extra = self.nc.sync.drain()
                    extra.ins.sync_info = mybir.SyncInfo(on_wait=[w], on_update=[])
            self.nc.all_engine_barrier()
            assert self.sems is not None
            popped = self.nc._tile_sem_poison_stack.pop()
            assert popped is self._sem_poison
            self.nc.clear_and_free_semaphores(list(self.sems.allocated().values()))
            self.nc.all_engine_barrier()
            _split_multi_waits(self.nc, mybir)

    TC = SplitDrainTileContext if fix_waits else tile.TileContext

    f32 = mybir.dt.float32
    f32r = mybir.dt.float32r
    bf = mybir.dt.bfloat16
    fp16 = mybir.dt.float16
    edt = mybir.dt.float8e4 if FP8_SCATTER else bf
    tpb = kt if FP8_SCATTER else 4 * kt   # tiles per 128-atom block
    twid = 128 if FP8_SCATTER else 32     # one-hot width
    tt = (NBLK if FP8_SCATTER else NWIN) * kt
    sc_blocks = list(SC_BLOCKS)

    nc = bass.Bass("TRN2", target_bir_lowering=False, debug=False, num_devices=num_devices)
    epos_d = nc.dram_tensor("epos", [128, tt * 6], f32, kind="ExternalInput")
    tmat_d = nc.dram_tensor("tmat", [128, tt * twid], edt, kind="ExternalInput")
    w1a_d = nc.dram_tensor("w1a", [128, HIDDEN], fp16, kind="ExternalInput")
    w1b_d = nc.dram_tensor("w1b", [33, HIDDEN], fp16, kind="ExternalInput")
    w2r_d = nc.dram_tensor("w2r", [128, 4 * HIDDEN], f32r, kind="ExternalInput")
    w3r_d = nc.dram_tensor("w3r", [128, 4], f32r, kind="ExternalInput")
    b2t_d = nc.dram_tensor("b2t", [128, 4], f32, kind="ExternalInput")
    b3s_d = nc.dram_tensor("b3s", [1, 1], f32, kind="ExternalInput")
    smat_d = nc.dram_tensor("smat", [EMAT_P, SC_BLK * 128], f32, kind="ExternalInput")
    shm_d = nc.dram_tensor("shm", [EMAT_P, SC_BLK * 128], f32, kind="ExternalInput")
    eout_d = nc.dram_tensor("eout", [1, 1], f32, kind="ExternalOutput")

    with TC(nc) as tc:
        with tc.tile_pool(name="const", bufs=1) as cpool:
            ident = cpool.tile([128, 128], f32)
            make_identity(nc, ident[:])
            ident_b = cpool.tile([128, 128], bf)
            nc.vector.tensor_copy(out=ident_b[:], in_=ident[:])
            identb = ident_b[:]
            negc = cpool.tile([128, 16], f32)
            for k in range(16):
                nc.vector.memset(negc[:, k:k + 1], float(-centers[k]))
            w1a_t = cpool.tile([128, HIDDEN], fp16)
            w1b_t = cpool.tile([33, HIDDEN], fp16)
            w2r_t = cpool.tile([128, 4 * HIDDEN], f32r)
            w3r_t = cpool.tile([128, 4], f32r)
            b2t_t = cpool.tile([128, 4], f32)
            b3s_t = cpool.tile([1, 1], f32)
            smat_t = cpool.tile([EMAT_P, SC_BLK * 128], f32)
            shm_t = cpool.tile([EMAT_P, SC_BLK * 128], f32)
            emat_t = cpool.tile([EMAT_P, SC_BLK * 128], f32)
            for dst, src in [(w1a_t, w1a_d), (w1b_t, w1b_d), (w2r_t, w2r_d),
                             (w3r_t, w3r_d), (b2t_t, b2t_d),
                             (b3s_t, b3s_d), (smat_t, smat_d), (shm_t, shm_d)]:
                nc.sync.dma_start(out=dst[:], in_=src.ap())
            nc.vector.memset(emat_t[:], 0.0)

            # ------------- fused pipeline over superchunks (4 blocks = 512 atoms)
            # The edge chain (dr -> r -> basis -> b*d) has ~11us of serial
            # latency woven across DVE/Act/Pool, so it is issued two
            # superchunks ahead of the scatter+MLP stage that consumes it
            # (eio/emid bufs=3 hold the in-flight superchunks).
            with tc.tile_pool(name="eio", bufs=3) as eio, \
                 tc.tile_pool(name="emid", bufs=3) as emid, \
                 tc.tile_pool(name="msc", bufs=2) as msc, \
                 tc.tile_pool(name="hmid", bufs=2) as hmid, \
                 tc.tile_pool(name="bio", bufs=2) as bio, \
                 tc.tile_pool(name="psA", bufs=3, space="PSUM") as psA_p, \
                 tc.tile_pool(name="psT", bufs=1, space="PSUM") as psT_p, \
                 tc.tile_pool(name="psB", bufs=3, space="PSUM") as psB_p:

                def emit_edge(ch, nblk, blk0):
                    nt = nblk * tpb                # edge tiles
                    t0 = blk0 * tpb

                    epos_t = emid.tile([128, nt, 6], f32, tag="epos")
                    tmat_t = eio.tile([128, nt, twid], edt, tag="tmat")
                    nc.sync.dma_start(out=epos_t[:].rearrange("p a b -> p (a b)"),
                                      in_=epos_d.ap()[:, t0 * 6:(t0 + nt) * 6])
                    nc.sync.dma_start(out=tmat_t[:].rearrange("p a b -> p (a b)"),
                                      in_=tmat_d.ap()[:, t0 * twid:(t0 + nt) * twid])

                    drt = emid.tile([128, nt, 3], f32, tag="drt")
                    sqt = emid.tile([128, nt, 3], f32, tag="sqt")
                    r2t = emid.tile([128, nt], f32, tag="r2t")
                    rt = emid.tile([128, nt], f32, tag="rt")
                    rit = emid.tile([128, nt], f32, tag="rit")
                    diff = emid.tile([128, nt, 16], f32, tag="diff", bufs=2)
                    feat_e = eio.tile([128, nt, 64], edt, tag="feate")

                    nc.vector.tensor_tensor(out=drt[:], in0=epos_t[:, :, 3:6],
                                            in1=epos_t[:, :, 0:3], op=mybir.AluOpType.subtract)
                    nc.vector.tensor_tensor(out=sqt[:], in0=drt[:], in1=drt[:],
                                            op=mybir.AluOpType.mult)
                    nc.vector.tensor_reduce(out=r2t[:], in_=sqt[:],
                                            axis=mybir.AxisListType.X, op=mybir.AluOpType.add)
                    nc.scalar.activation(out=rt[:], in_=r2t[:],
                                         func=mybir.ActivationFunctionType.Sqrt)
                    nc.vector.tensor_scalar(out=r2t[:], in0=rt[:], scalar1=EPS, scalar2=None,
                                            op0=mybir.AluOpType.add)
                    nc.vector.reciprocal(out=rit[:], in_=r2t[:])
                    nc.vector.tensor_tensor(out=drt[:], in0=drt[:],
                                            in1=rit[:].unsqueeze(2).broadcast_to([128, nt, 3]),
                                            op=mybir.AluOpType.mult)
                    # diff[p,t,k] = r - c_k; square in place on Pool
                    nc.gpsimd.tensor_tensor(
                        out=diff[:],
                        in0=rt[:].unsqueeze(2).broadcast_to([128, nt, 16]),
                        in1=negc[:].unsqueeze(1).broadcast_to([128, nt, 16]),
                        op=mybir.AluOpType.add)
                    nc.gpsimd.tensor_tensor(out=diff[:], in0=diff[:], in1=diff[:],
                                            op=mybir.AluOpType.mult)
                    nc.scalar.activation(out=feat_e[:, :, 0:16], in_=diff[:],
                                         func=mybir.ActivationFunctionType.Exp,
                                         scale=float(neg_beta))
                    bdout = feat_e[:, :, 16:64].rearrange("p t (c k) -> p t c k", c=3)
                    bview = feat_e[:, :, 0:16].unsqueeze(2).broadcast_to([128, nt, 3, 16])
                    nc.vector.tensor_tensor(
                        out=bdout[:],
                        in0=bview,
                        in1=drt[:].unsqueeze(3).broadcast_to([128, nt, 3, 16]),
                        op=mybir.AluOpType.mult)
                    return tmat_t, feat_e

                def emit_scatter(ch, nblk, blk0, tmat_t, feat_e):
                    nt = nblk * tpb
                    w = nblk * 128
                    # scatter + moment extraction (batched per superchunk)
                    m1e = msc.tile([128, nblk, 3, 24], bf, tag="m1e")
                    feat_at = msc.tile([128, nblk, NFEAT], bf, tag="feat_at")
                    nc.vector.memset(feat_at[:, :, 160:161], 1.0)
                    psA = psA_p.tile([128, nblk, 64], f32, tag="psA")
                    if FP8_SCATTER:
                        # block-level one-hot, DoubleRow over tile pairs
                        for b in range(nblk):
                            for kp in range(tpb // 2):
                                ti = b * tpb + 2 * kp
                                nc.tensor.matmul(
                                    out=psA[:, b, :],
                                    lhsT=tmat_t[:, ti:ti + 2, :],
                                    rhs=feat_e[:, ti:ti + 2, :],
                                    start=(kp == 0), stop=(kp == tpb // 2 - 1),
                                    perf_mode=mybir.MatmulPerfMode.DoubleRow,
                                    skip_group_check=True)
                    else:
                        for b in range(nblk):
                            for wv in range(4):
                                for k in range(kt):
                                    ti = (b * 4 + wv) * kt + k
                                    nc.tensor.matmul(
                                        out=psA[32 * wv:32 * wv + 32, b, :],
                                        lhsT=tmat_t[:, ti, :], rhs=feat_e[:, ti, :],
                                        start=(k == 0), stop=(k == kt - 1),
                                        tile_position=(0, 32 * wv), skip_group_check=True)
                    nc.vector.tensor_copy(out=feat_at[:, :, 0:16], in_=psA[:, :, 0:16])
                    psA_ck = psA[:, :, 16:64].rearrange("p b (c k) -> p b c k", c=3)
                    nc.vector.tensor_copy(out=m1e[:, :, :, 0:16], in_=psA_ck)
                    nc.vector.tensor_copy(out=m1e[:, :, :, 16:24], in_=psA_ck[:, :, :, 0:8])

                    # j-outer symmetric contraction: G1[j,k1] = sum_d m1[d,k1]*m1[d,k1+j]
                    g1a = msc.tile([128, nblk, 9, 16], bf, tag="g1a")
                    g1b = msc.tile([128, nblk, 9, 16], bf, tag="g1b")
                    g1c = msc.tile([128, nblk, 9, 16], bf, tag="g1c")
                    for d, dst in ((0, g1a), (1, g1b), (2, g1c)):
                        nc.vector.tensor_tensor(
                            out=dst[:],
                            in0=m1e[:, :, d, 0:16].unsqueeze(2).broadcast_to([128, nblk, 9, 16]),
                            in1=_diag_view_j(m1e, d, nblk),
                            op=mybir.AluOpType.mult)
                    nc.vector.tensor_tensor(out=g1a[:], in0=g1a[:], in1=g1b[:],
                                            op=mybir.AluOpType.add)
                    nc.vector.tensor_tensor(
                        out=feat_at[:, :, 16:160].rearrange("p b (j k) -> p b j k", j=9),
                        in0=g1a[:], in1=g1c[:], op=mybir.AluOpType.add)
                    return feat_at

                def emit_mlp(ch, nblk, blk0, feat_at):
                    w = nblk * 128
                    # transpose features to [feat, atoms]
                    psT0 = psT_p.tile([128, nblk, 128], bf, tag="psT0")
                    psT1 = psT_p.tile([33, nblk, 128], bf, tag="psT1")
                    for b in range(nblk):
                        nc.tensor.matmul(out=psT0[:, b, :], lhsT=feat_at[:, b, 0:128],
                                         rhs=identb, is_transpose=True,
                                         start=True, stop=True, skip_group_check=True)
                        nc.tensor.matmul(out=psT1[:, b, :], lhsT=feat_at[:, b, 128:161],
                                         rhs=identb, is_transpose=True,
                                         start=True, stop=True, skip_group_check=True)
                    fst0 = msc.tile([128, nblk, 128], fp16, tag="fst0")
                    fst1 = msc.tile([33, nblk, 128], fp16, tag="fst1")
                    nc.vector.tensor_copy(out=fst0[:], in_=psT0[:])
                    nc.vector.tensor_copy(out=fst1[:], in_=psT1[:])

                    # MLP in 512-atom sub-chunks
                    for s0 in range(0, nblk, 4):
                        sb = min(4, nblk - s0)
                        ws = sb * 128
                        f0c = fst0[:, s0:s0 + sb, :].rearrange("p b x -> p (b x)")
                        f1c = fst1[:, s0:s0 + sb, :].rearrange("p b x -> p (b x)")
                        h1s = hmid.tile([128, 4, ws], f32r, tag="h1s")
                        for h in range(4):
                            ps = psB_p.tile([128, ws], f32, tag="psB")
                            nc.tensor.matmul(out=ps[:], lhsT=w1a_t[:, h * 128:(h + 1) * 128],
                                             rhs=f0c, start=True, stop=False, skip_group_check=True)
                            nc.tensor.matmul(out=ps[:], lhsT=w1b_t[:, h * 128:(h + 1) * 128],
                                             rhs=f1c, start=False, stop=True, skip_group_check=True)
                            nc.scalar.activation(out=h1s[:, h, :], in_=ps[:],
                                                 func=mybir.ActivationFunctionType.Silu)
                        h2s = hmid.tile([128, 4, ws], f32r, tag="h2s")
                        for h in range(4):
                            ps = psB_p.tile([128, ws], f32, tag="psB")
                            for k in range(4):
                                nc.tensor.matmul(
                                    out=ps[:], lhsT=w2r_t[:, k * HIDDEN + h * 128:k * HIDDEN + (h + 1) * 128],
                                    rhs=h1s[:, k, :], start=(k == 0), stop=(k == 3), skip_group_check=True)
                            nc.scalar.activation(out=h2s[:, h, :], in_=ps[:],
                                                 func=mybir.ActivationFunctionType.Silu,
                                                 bias=b2t_t[:, h:h + 1])
                        pse = psB_p.tile([128, ws], f32, tag="psB")
                        for k in range(4):
                            nc.tensor.matmul(out=pse[0:1, :], lhsT=w3r_t[:, k:k + 1],
                                             rhs=h2s[:, k, :], start=(k == 0), stop=(k == 3), skip_group_check=True)
                        erow = bio.tile([1, ws], f32, tag="erow")
                        nc.scalar.activation(out=erow[:], in_=pse[0:1, :],
                                             func=mybir.ActivationFunctionType.Identity,
                                             bias=b3s_t[:])
                        nc.sync.dma_start(out=emat_t[ch:ch + 1, s0 * 128:s0 * 128 + ws],
                                          in_=erow[:])

                # driver: three-stage software pipeline. Edge chains run two
                # superchunks ahead of their scatter; transposes+MLP trail the
                # scatter by one superchunk so the g1 contraction has a full
                # period of slack before the transposes consume it.
                blk0s = []
                b0 = 0
                for nblk in sc_blocks:
                    blk0s.append(b0)
                    b0 += nblk
                n_sc = len(sc_blocks)
                edges = {}
                feats = {}

                def do_edge(u):
                    edges[u] = emit_edge(u, sc_blocks[u], blk0s[u])

                def do_scatter(u):
                    tm, fe = edges.pop(u)
                    feats[u] = emit_scatter(u, sc_blocks[u], blk0s[u], tm, fe)

                def do_mlp(u):
                    emit_mlp(u, sc_blocks[u], blk0s[u], feats.pop(u))

                for rep in range(reps):
                    for u in range(3):
                        do_edge(u)
                    do_scatter(0)
                    for u in range(1, n_sc):
                        do_mlp(u - 1)
                        if u + 2 < n_sc:
                            do_edge(u + 2)
                        do_scatter(u)
                    do_mlp(n_sc - 1)

                # final readout
                u = bio.tile([EMAT_P, SC_BLK * 128], f32, tag="u", bufs=1)
                acc = bio.tile([EMAT_P, 1], f32, tag="acc", bufs=1)
                nc.vector.tensor_tensor(out=u[:], in0=emat_t[:], in1=smat_t[:],
                                        op=mybir.AluOpType.mult)
                nc.vector.scalar_tensor_tensor(out=u[:], in0=u[:], scalar=1.0, in1=shm_t[:],
                                               op0=mybir.AluOpType.mult,
                                               op1=mybir.AluOpType.add, accum_out=acc[:])
                ones = bio.tile([EMAT_P, 1], f32, tag="ones", bufs=1)
                nc.vector.memset(ones[:], 1.0)
                psf = psB_p.tile([128, 1], f32, tag="psB")
                nc.tensor.matmul(out=psf[0:1, :], lhsT=ones[:], rhs=acc[:], start=True,
                                 stop=True, skip_group_check=True)
                eo = bio.tile([1, 1], f32, tag="eo", bufs=1)
                nc.scalar.activation(out=eo[:], in_=psf[0:1, :],
                                     func=mybir.ActivationFunctionType.Copy)
                nc.sync.dma_start(out=eout_d.ap(), in_=eo[:])

    _PROG_CACHE[key] = nc
    return nc


def _diag_view_j(m1e, d, nblk):
    """AP [128, nblk, j(9), k1(16)] reading m1e[:, :, d, j + k1] (both trailing
    dims stride 1 -> packed operand, eligible for the DVE 2x mode)."""
    import concourse.ap as cap

    base = m1e[:, :, d, :]                         # [128, nblk, 24]
    v = base.unsqueeze(2)                          # [128, nblk, 1, 24]
    v = v[:, :, :, 0:16]                           # [128, nblk, 1, 16]
    v = v.broadcast_to([128, nblk, 9, 16])         # j dim stride 0
    apl = [list(p) for p in v.ap]
    apl[-2] = [1, 9]
    return cap.AP(v.tensor, v.offset, apl, v.const_val, v.runtime_checks,
                  v.dep_tracking_offset)


LAST_EXEC_NS = None
PROFILE = False


def kernel(**inputs):
    from concourse.bass_utils import run_bass_kernel_spmd

    consts, per_core, shared = _preprocess(**inputs)
    nc = _get_program(consts["kt"], consts["centers"], consts["neg_beta"],
                      float(np.asarray(shared["b3s"]).reshape(())))
    in_maps = []
    for c in range(N_CORES):
        m = dict(
            epos=per_core["epos"][c],
            tmat=per_core["tmat"][c],
            w1a=shared["w1a"], w1b=shared["w1b"],
            w2r=shared["w2r"], w3r=shared["w3r"],
            b2t=shared["b2t"], b3s=shared["b3s"],
            smat=per_core["smat"][c], shm=per_core["shmat"][c],
        )
        in_maps.append(m)
    global LAST_EXEC_NS
    kwargs = {}
    if PROFILE:
        import tempfile
        kwargs = dict(trace=True, tmpdir=tempfile.mkdtemp(prefix="ktrace_"))
    res = run_bass_kernel_spmd(nc, in_maps, core_ids=list(range(N_CORES)), **kwargs)
    if getattr(res, "exec_time_ns", None):
        LAST_EXEC_NS = res.exec_time_ns
    if PROFILE:
        globals()["LAST_RESULTS"] = res
    total = np.float32(0.0)
    for c in range(N_CORES):
        total += np.float32(res.results[c]["eout"].reshape(()))
    return np.float32(total)
